# revision 17
# baseline (speedup 1.0000x reference)
"""FCPlanenet Trainium2 kernel (8-core data-parallel over batch).

Network (per batch of T=8192 points, feature-major [feat, T] on chip):
  net0 = p @ Wpos + bpos            [256, T]   (K=3 matmul, quad-packed)
  net1 = relu(net0) @ W0 + b0       [128, T]   (K=256)
  netk+1 = relu(cat(netk, max_t netk)) @ Wk + bk   for W1..W3
  out = MLP head over max_t net4    [9] per batch

The pooled-concat half is rank-1 (same vector at every point), so each layer
reduces to Wk_a.T @ relu(netk) plus a per-feature constant vector C applied
at drain time.  Big matmuls run in bf16 (weights + activations; ~1e-3 final
rel err, gate is 2e-2).  PSUM-touching elementwise work is split between two
engines (gpsimd cannot access PSUM):
  - ACT:  true-relu drains  r = relu(psum + C_R)          (R-class chunks)
          plus identity drains of some D-stage psums to bf16 scratch
  - DVE:  tensor_scalar+accum drains  r~ = max(psum, -C_T)  (= relu - C_T)
          emitting the per-chunk pooled max in the same pass (T-class)
Pooling coverage of ACT-drained bf16 data costs almost nothing: one DVE
tensor_scalar+accum in 4x mode (0.26 ns/elem, all-SBUF 2-byte).  gpsimd takes
the tiny [128,1] boundary constant ops.  Offset-class constants fold into the
next layer's constants via tiny matvecs (Wa @ C_T, off the critical path).
"""

import os
import sys

import numpy as np

for _p in ("/opt/trn_rl_repo", "/root/.axon_site/_ro/trn_rl_repo"):
    if os.path.isdir(_p) and _p not in sys.path:
        sys.path.insert(0, _p)

import concourse.bass as bass  # noqa: E402
import concourse.tile as tile  # noqa: E402
from concourse import bacc, mybir  # noqa: E402
from concourse.bass_utils import run_bass_kernel_spmd  # noqa: E402

F32 = mybir.dt.float32
F32R = mybir.dt.float32r
BF16 = mybir.dt.bfloat16
AX = mybir.AxisListType.X
MAX = mybir.AluOpType.max
ADD = mybir.AluOpType.add
MUL = mybir.AluOpType.mult
RELU = mybir.ActivationFunctionType.Relu
IDENT = mybir.ActivationFunctionType.Identity

NCORES = 8
B = 32
T = 8192
BPC = B // NCORES  # batches per core
NQ = 4             # point quads (for K=3 matmul row-packing)
QT = T // NQ       # 2048 points per quad
NCH = 512          # matmul free-dim chunk (one PSUM bank)
NSUP = 1024        # drain supertile (2 chunks)
NST = T // NSUP    # 8 supertiles per layer

NEG_INF = -1.0e30

# bias tile columns
BC_BPOS_A, BC_BPOS_B = 0, 1
BC_B0, BC_B1, BC_B2, BC_B3 = 2, 3, 4, 5
BC_BC, BC_BM0, BC_BM1, BC_BM2, BC_BP = 6, 7, 8, 9, 10
BC_NEG_B0 = 11
BC_NEG_B1, BC_NEG_B2, BC_NEG_B3 = 12, 13, 14

# wm tile blocks of 128 cols: w0a w0b w1a w1b w2a w2b w3a w3b wc wm0 wm1 wm2 wp
WM_COLS = 13 * 128 + 16

# ---- engine assignment tables (tuning knobs) ----
# L-layer supertiles 0..7: 'A' = ACT true-relu (R-class, bf16 out, covered by
# one DVE 4x accum pass), 'V' = DVE ts+accum offset form (T-class).
# R-sts contiguous and executed interleaved with T via L_ORDER; the last
# executed st is a T-st so the boundary chain gates on DVE-internal work.
L_ENG = ("A", "A", "A", "V", "V", "V", "V", "V")
L_RSTS = (0, 1, 2)           # R-class sts (contiguous), cov -> acc col 0
L_ACC_T = {3: 1, 4: 2, 5: 3, 6: 4, 7: 5}
L_NT = 5  # number of T accum cols (starting at col 1)
# execution order: T-sts 3,4 first (their chunks were drained earliest in the
# previous layer), R interleaved, T-st 7 last
L_ORDER = (3, 4, 0, 5, 1, 6, 2, 7)

# pos supertile tasks idx=8*qp+2*s+h: 'A' = ACT, 'V' = DVE (true relu both).
# Batch 0 has no other work to overlap, so its pos spreads onto DVE too.
POS_ENG = ("A",) * 16
POS_ENG_PRO = tuple("V" if i in (2, 5, 8, 11, 13, 15) else "A"
                    for i in range(16))

# D stage: sts in D_ACT drain via ACT-ident to bf16 scratch (one DVE 4x
# accum covers them); the rest are DVE ts+accum psum singles.
# r3 chunk classes follow L_RSTS: sts 0,1,2 are R (C_R^D), rest T (C_T^D).
D_ACT = (4, 5)               # must be same-class (T)
D_COL = {0: 0, 1: 1, 2: 2, 3: 3, "act": 4, 6: 5, 7: 6}
D_RCOL = (0, 3)              # accD cols [0,3) are R-class raw maxes
D_TCOL = (3, 7)              # accD cols [3,7) are T-class raw maxes


def _f32r(ap):
    return ap if ap.dtype == F32R else ap.bitcast(F32R)


def _f32(ap):
    return ap if ap.dtype == F32 else ap.bitcast(F32)


def _emit_core_program(tc, nc, pt_d, wpos_d, wm16_d, wmf_d, wmn_d, bias_d,
                       out_d, reps=1):
    ctx_pools = []

    def pool(name, bufs, space="SBUF"):
        p = tc.alloc_tile_pool(name=name, bufs=bufs, space=space)
        ctx_pools.append(p)
        return p

    const = pool("const", 1)
    ptp = pool("ptp", 2)
    r0p = pool("r0p", 1)
    netp = pool("netp", 2)
    smallp = pool("smallp", 8)
    vecp = pool("vecp", 30)
    covp = pool("covp", 2)
    dscp = pool("dscp", 2)
    headp = pool("headp", 2)
    psmm = pool("psmm", 3, space="PSUM")
    psvp = pool("psvp", 2, space="PSUM")

    # ---- constants ----
    bias_sb = const.tile([128, 16], F32, name="bias_sb")
    nc.sync.dma_start(bias_sb[:], bias_d)
    wpos_sb = const.tile([99, 256], BF16, name="wpos_sb")
    for q in range(2):
        nc.sync.dma_start(wpos_sb[32 * q:32 * q + 3, :], wpos_d[q])
    _wpos_rest = [False]

    def load_wpos_rest():
        if not _wpos_rest[0]:
            for q in range(2, NQ):
                nc.sync.dma_start(wpos_sb[32 * q:32 * q + 3, :], wpos_d[q])
            _wpos_rest[0] = True
    wm_sb = const.tile([128, WM_COLS], BF16, name="wm_sb")
    wmf_sb = const.tile([128, WM_COLS], F32, name="wmf_sb")
    wmn_sb = const.tile([128, 384], F32, name="wmn_sb")
    _wm_loaded = [False]

    def load_wm():
        if not _wm_loaded[0]:
            # W0 blocks first: L0 matmuls need them ~1us in; the f32 matvec
            # copy is only needed at the first boundary.
            nc.sync.dma_start(wm_sb[:, 0:256], wm16_d[:, 0:256])
            nc.sync.dma_start(wm_sb[:, 256:WM_COLS], wm16_d[:, 256:WM_COLS])
            nc.sync.dma_start(wmn_sb[:, 0:384], wmn_d[:, 0:384])
            nc.sync.dma_start(wmf_sb[:, 0:WM_COLS], wmf_d[:, 0:WM_COLS])
            _wm_loaded[0] = True

    def W(i):       # bf16 weights for the big matmuls
        return wm_sb[:, 128 * i:128 * (i + 1)]

    def Wf(i):      # f32 weights for [128,1] matvecs
        return wmf_sb[:, 128 * i:128 * (i + 1)]

    def negWb(li):  # f32 -W1b/-W2b/-W3b for the negated boundary chain
        return wmn_sb[:, 128 * li:128 * (li + 1)]

    def bcol(i):
        return bias_sb[:, i:i + 1]

    def mk_acc(name):
        """Accum tile, initialized to -inf: the HW tensor_scalar accum_out
        read-modify-writes the destination."""
        acc = smallp.tile([128, 8], F32, tag="pp", name=name)
        nc.gpsimd.memset(acc[:], NEG_INF)
        return acc

    def pos_tasks(b, pt_sb, r0):
        """16 supertile tasks for the pos layer of batch b (true relu)."""
        eng = POS_ENG_PRO if b == 0 else POS_ENG
        tasks = []
        for qp in range(2):
            for s in range(4):
                for h in range(2):
                    def t(qp=qp, s=s, h=h):
                        ps = psmm.tile([128, NSUP], F32, tag="mm", name="ps_pos")
                        for jq in range(2):
                            q = 2 * qp + jq
                            nc.tensor.matmul(
                                ps[:, NCH * jq:NCH * (jq + 1)],
                                wpos_sb[32 * q:32 * q + 3, 128 * h:128 * (h + 1)],
                                pt_sb[32 * q:32 * q + 3, NCH * s:NCH * (s + 1)],
                                start=True, stop=True,
                                tile_position=(32 * q, 0),
                            )
                        g0 = 8 * qp + s
                        dst = (r0[h].rearrange("p (g c) -> p g c", c=NCH)
                               [:, g0:g0 + 5:4, :])
                        srcv = ps.rearrange("p (g c) -> p g c", c=NCH)
                        idx = 8 * qp + 2 * s + h
                        if eng[idx] == "V":
                            nc.vector.tensor_scalar(dst, srcv, bcol(BC_BPOS_A + h),
                                                    0.0, op0=ADD, op1=MAX)
                        else:
                            nc.scalar.activation(dst, srcv, RELU,
                                                 bias=bcol(BC_BPOS_A + h))
                    tasks.append(t)
        return tasks

    def layer_tasks(li, b, r0, r_prev, r_out, acc, consts_box):
        """(mm, drain) task pairs for pooled layer li (0..2), in L_ORDER.
        consts_box[0] = (cR, cT, negT) thunks, filled by the boundary that
        runs between the first mms and the drains.  acc: [128,8] f32."""
        cR = lambda: consts_box[0][0]()
        negT = lambda: consts_box[0][2]()
        ps_box = {}
        drained_R = set()

        def emit_mm(st, li):
            ps = psmm.tile([128, NSUP], F32, tag="mm", name=f"ps_l{li}")
            ps_box[st] = ps
            for j in range(2):
                c = 2 * st + j
                osl = ps[:, NCH * j:NCH * (j + 1)]
                csl = slice(NCH * c, NCH * (c + 1))
                if li == 0:
                    nc.tensor.matmul(osl, W(0), r0[0][:, csl],
                                     start=True, stop=False)
                    nc.tensor.matmul(osl, W(1), r0[1][:, csl],
                                     start=False, stop=True)
                else:
                    nc.tensor.matmul(osl, W(2 * li), r_prev[:, csl],
                                     start=True, stop=True)

        def emit_drain(st, li):
            ps = ps_box.pop(st)
            dsl = slice(NSUP * st, NSUP * (st + 1))
            if L_ENG[st] == "A":
                nc.scalar.activation(r_out[:, dsl], ps[:], RELU, bias=cR())
                drained_R.add(st)
                if drained_R == set(L_RSTS):
                    cov = covp.tile([128, len(L_RSTS) * NSUP], BF16, tag="cov",
                                    name="cov")
                    nc.vector.tensor_scalar(cov[:],
                                            r_out[:, 0:len(L_RSTS) * NSUP],
                                            NEG_INF, NEG_INF, op0=MAX, op1=MAX,
                                            accum_out=acc[:, 0:1])
            else:
                col = L_ACC_T[st]
                nc.vector.tensor_scalar(r_out[:, dsl], ps[:], negT(), NEG_INF,
                                        op0=MAX, op1=MAX,
                                        accum_out=acc[:, col:col + 1])

        return [(lambda st=st, li=li: emit_mm(st, li),
                 lambda st=st, li=li: emit_drain(st, li)) for st in L_ORDER]

    def run_layer(pairs, fillers=(), boundary_fn=None):
        """Emit a layer: first two sts' mms, then the previous boundary's
        matvec chain, then the remaining tasks with fillers interleaved."""
        fl = list(fillers)
        pairs[0][0]()
        pairs[1][0]()
        out = None
        if boundary_fn is not None:
            out = boundary_fn()
        pairs[0][1]()
        pairs[1][1]()
        rest = [lambda p=p: (p[0](), p[1]()) for p in pairs[2:]]
        for t in interleave(rest, fl):
            t()
        return out

    def d_tasks(b, r_prev, accD, last=False):
        """D-stage supertiles: matmuls + raw psum maxes into accD.
        Returns (rcol_range, tcol_range) for the final reduces.  The last
        batch has no pos fillers for ACT, so it drains everything via
        ACT-ident + two DVE 4x covs instead of DVE psum singles."""
        tasks = []
        if last:
            groups = {"T": (1, 2, 4, 5)}
            gcol = {"T": 4}
            singles = {0: 0, 3: 1, 6: 2, 7: 3}
            ranges = ((0, 3), (3, 5))
        else:
            groups = {"T": D_ACT}
            gcol = {"T": D_COL["act"]}
            singles = {st: D_COL[st] for st in range(NST)
                       if st not in D_ACT}
            ranges = (D_RCOL, D_TCOL)
        dscr = {g: dscp.tile([128, len(sts) * NSUP], BF16, tag=f"dsc{g}",
                             name=f"dscr{g}_{b}")
                for g, sts in groups.items()}
        member = {st: (g, k) for g, sts in groups.items()
                  for k, st in enumerate(sts)}

        def emit_st(st):
            ps = psmm.tile([128, NSUP], F32, tag="mm", name="ps_d")
            for j in range(2):
                c = 2 * st + j
                csl = slice(NCH * c, NCH * (c + 1))
                nc.tensor.matmul(ps[:, NCH * j:NCH * (j + 1)], W(6),
                                 r_prev[:, csl], start=True, stop=True)
            if st in member:
                g, k = member[st]
                nc.scalar.activation(dscr[g][:, NSUP * k:NSUP * (k + 1)],
                                     ps[:], IDENT, bias=0.0)
                if st == groups[g][-1]:
                    cov = covp.tile([128, len(groups[g]) * NSUP], BF16,
                                    tag="cov", name=f"covd{g}")
                    col = gcol[g]
                    nc.vector.tensor_scalar(cov[:], dscr[g][:], NEG_INF,
                                            NEG_INF, op0=MAX, op1=MAX,
                                            accum_out=accD[:, col:col + 1])
            else:
                col = singles[st]
                scrj = headp.tile([128, NSUP], BF16, tag="scrj", name="scrj")
                nc.vector.tensor_scalar(scrj[:], ps[:], NEG_INF, NEG_INF,
                                        op0=MAX, op1=MAX,
                                        accum_out=accD[:, col:col + 1])

        for st in range(NST):
            tasks.append(lambda st=st: emit_st(st))
        return tasks, ranges

    def interleave(a, bl):
        out = []
        n = max(len(a), len(bl))
        for i in range(n):
            if i < len(a):
                out.append(a[i])
            if i < len(bl):
                out.append(bl[i])
        return out

    import contextlib

    def _rep_scope():
        if reps > 1:
            return tc.For_i(0, reps, 1,
                            hint_engines=(mybir.EngineType.PE,
                                          mybir.EngineType.Activation,
                                          mybir.EngineType.DVE,
                                          mybir.EngineType.Pool))
        return contextlib.nullcontext()

    with _rep_scope():
        # per-batch state created lazily
        def new_batch_state(b):
            pt_sb = ptp.tile([99, QT], BF16, tag="pt", name="pt_sb")
            for q in range(NQ):
                nc.sync.dma_start(pt_sb[32 * q:32 * q + 3, :], pt_d[b, q])
            r0a = r0p.tile([128, T], BF16, tag="r0a", name="r0a")
            r0b = r0p.tile([128, T], BF16, tag="r0b", name="r0b")
            return pt_sb, (r0a, r0b)

        def mk_consts_L0():
            # C_R = C_T = b0 (pos chunks are all true-relu)
            return (lambda: bcol(BC_B0), lambda: bcol(BC_B0),
                    lambda: bcol(BC_NEG_B0))

        def boundary(li, b, acc, cur):
            """After layer li (0..2): compute m and next-layer constants from
            the finished layer's consts `cur` = (cR, cT, negT) thunks.
            Critical chain is 3 sem hops: reduce+stt (DVE) -> psvN (PE) ->
            negT stt (DVE).  acc col 0 = max_t relu(net) over R-chunks
            (final form); cols 1..L_NT = max(max_t psum, -C_T) (T-chunks)."""
            wa_i = 2 * (li + 1)
            bc_i = BC_B1 + li
            nbc_i = BC_NEG_B1 + li
            cT_cur, negT_cur = cur[1](), cur[2]()
            # pooled max m = max(reduce(accT) + C_T, accR)
            mT = vecp.tile([128, 1], F32, tag="v", name=f"mT{li}_{b}")
            nc.vector.tensor_reduce(mT, acc[:, 1:1 + L_NT], AX, MAX)
            m = vecp.tile([128, 1], F32, tag="v", name=f"m{li}_{b}")
            nc.vector.scalar_tensor_tensor(m, mT, cT_cur, acc[:, 0:1],
                                           op0=ADD, op1=MAX)
            # psv2N = Wa @ (-C_T) (issued early, off the m critical path)
            psv2 = psvp.tile([128, 1], F32, tag="psv", name=f"psv2_{li}_{b}")
            nc.tensor.matmul(psv2[:], Wf(wa_i), negT_cur, start=True, stop=True)
            psv2s = vecp.tile([128, 1], F32, tag="v", name=f"p2s{li}_{b}")
            nc.vector.tensor_scalar(psv2s, psv2[:], 0.0, 0.0, op0=ADD, op1=ADD)
            # psvN = (-Wb) @ m
            psv = psvp.tile([128, 1], F32, tag="psv", name=f"psv_{li}_{b}")
            nc.tensor.matmul(psv[:], negWb(li), m, start=True, stop=True)
            # negT' = (psvN + (-b)) + psv2N  = -(Wb m + b + Wa C_T)
            negT = vecp.tile([128, 1], F32, tag="v", name=f"nT{li}_{b}")
            nc.vector.scalar_tensor_tensor(negT, psv[:], bcol(nbc_i), psv2s,
                                           op0=ADD, op1=ADD)
            # positive forms, off the critical path
            cR = vecp.tile([128, 1], F32, tag="v", name=f"cR{li}_{b}")
            nc.scalar.activation(cR, psv[:], IDENT, bias=bcol(bc_i), scale=-1.0)
            cT = vecp.tile([128, 1], F32, tag="v", name=f"cT{li}_{b}")
            nc.gpsimd.tensor_scalar(cT, negT, -1.0, 0.0, op0=MUL, op1=ADD)
            consts = (lambda: cR, lambda: cT, lambda: negT)
            return consts, cT

        # prologue: pos(0) interleaved with L0(0) in chunk-ready order
        st0 = new_batch_state(0)
        load_wpos_rest()
        load_wm()
        states = {0: st0}
        p0 = pos_tasks(0, st0[0], st0[1])
        cb0 = [mk_consts_L0()]
        r1_0 = netp.tile([128, T], BF16, tag="net", name="r1_0")
        acc0_0 = mk_acc("a0_0")
        l0_0 = layer_tasks(0, 0, st0[1], None, r1_0, acc0_0, cb0)
        # l0_0 pairs are in L_ORDER; map st -> pair
        by_st = {st: l0_0[i] for i, st in enumerate(L_ORDER)}

        def do(st):
            by_st[st][0]()
            by_st[st][1]()
        for t in p0[0:4]:
            t()
        do(0); do(2)
        for t in p0[4:8]:
            t()
        do(1); do(3)
        for t in p0[8:12]:
            t()
        do(4); do(6)
        for t in p0[12:16]:
            t()
        do(5); do(7)

        for b in range(BPC):
            _, r0 = states[b]

            if b == 0:
                r1, acc0, consts0 = r1_0, acc0_0, cb0[0]
            else:
                consts0 = mk_consts_L0()
                r1 = netp.tile([128, T], BF16, tag="net", name=f"r1_{b}")
                acc0 = mk_acc(f"a0_{b}")
                for mm, dr in layer_tasks(0, b, r0, None, r1, acc0,
                                          [consts0]):
                    mm(); dr()

            filler = []
            if b + 1 < BPC:
                stn = new_batch_state(b + 1)
                states[b + 1] = stn
                filler = pos_tasks(b + 1, stn[0], stn[1])

            def mk_bnd(cb, li, acc_, cur_):
                def f():
                    res = boundary(li, b, acc_, cur_)
                    cb[0] = res[0]
                    return res
                return f

            r2 = netp.tile([128, T], BF16, tag="net", name=f"r2_{b}")
            acc1 = mk_acc(f"a1_{b}")
            cb1 = [None]
            res = run_layer(layer_tasks(1, b, None, r1, r2, acc1, cb1),
                            filler[0:5], mk_bnd(cb1, 0, acc0, consts0))
            consts1, cT1 = res

            r3 = netp.tile([128, T], BF16, tag="net", name=f"r3_{b}")
            acc2 = mk_acc(f"a2_{b}")
            cb2 = [None]
            res = run_layer(layer_tasks(2, b, None, r2, r3, acc2, cb2),
                            filler[5:10], mk_bnd(cb2, 1, acc1, consts1))
            consts2, cT2 = res

            accD = mk_acc(f"aD_{b}")
            dts, (rcols, tcols) = d_tasks(b, r3, accD, last=(b + 1 == BPC))
            dts[0](); dts[1]()
            constsD, cTD = boundary(2, b, acc2, consts2)
            cRD = constsD[0]()
            for t in interleave(dts[2:], filler[10:16]):
                t()

            # s = relu(max(reduce(accD_T) + C_T^D, reduce(accD_R) + C_R^D))
            # (all DVE: engine-internal ordering, no cross-engine sem hops)
            sR0 = vecp.tile([128, 1], F32, tag="v", name=f"sR0_{b}")
            nc.vector.tensor_reduce(sR0, accD[:, rcols[0]:rcols[1]], AX, MAX)
            sRc = vecp.tile([128, 1], F32, tag="v", name=f"sRc_{b}")
            nc.vector.tensor_scalar(sRc, sR0, cRD, 0.0, op0=ADD, op1=ADD)
            sT0 = vecp.tile([128, 1], F32, tag="v", name=f"sT0_{b}")
            nc.vector.tensor_reduce(sT0, accD[:, tcols[0]:tcols[1]], AX, MAX)
            spre = vecp.tile([128, 1], F32, tag="v", name=f"sp_{b}")
            nc.vector.scalar_tensor_tensor(spre, sT0, cTD, sRc,
                                           op0=ADD, op1=MAX)
            s_b = vecp.tile([128, 1], F32, tag="v", name=f"s_{b}")
            nc.vector.tensor_scalar(s_b, spre, 0.0, 0.0, op0=MAX, op1=ADD)

            # incremental head for this batch (all tiny fp32 ops)
            hb = s_b
            for wi, bi in ((8, BC_BC), (9, BC_BM0), (10, BC_BM1), (11, BC_BM2)):
                ps = psvp.tile([128, 1], F32, tag="psv", name=f"ph{wi}_{b}")
                nc.tensor.matmul(ps[:], Wf(wi), hb[:], start=True, stop=True)
                h2 = vecp.tile([128, 1], F32, tag="v", name=f"h{wi}_{b}")
                nc.scalar.activation(h2, ps[:], RELU, bias=bcol(bi))
                hb = h2
            ps9 = psvp.tile([9, 1], F32, tag="psv", name=f"po_{b}")
            nc.tensor.matmul(ps9[:], wmf_sb[:, 1536:1536 + 9], hb[:],
                             start=True, stop=True)
            ob = headp.tile([9, 1], F32, tag="o", name=f"ob_{b}")
            nc.scalar.activation(ob, ps9[:], IDENT, bias=bias_sb[0:9, BC_BP:BC_BP + 1])
            nc.sync.dma_start(out_d[b:b + 1].rearrange("b f -> f b"), ob[:])

    for p in reversed(ctx_pools):
        p.release()


def build_program(reps=1):
    nc = bacc.Bacc("TRN2", target_bir_lowering=False, debug=False,
                   num_devices=NCORES)
    pt_d = nc.dram_tensor("pt", [BPC, NQ, 3, QT], BF16, kind="ExternalInput").ap()
    wpos_d = nc.dram_tensor("wpos", [NQ, 3, 256], BF16, kind="ExternalInput").ap()
    wm16_d = nc.dram_tensor("wm16", [128, WM_COLS], BF16, kind="ExternalInput").ap()
    wmf_d = nc.dram_tensor("wmf", [128, WM_COLS], F32, kind="ExternalInput").ap()
    wmn_d = nc.dram_tensor("wmn", [128, 384], F32, kind="ExternalInput").ap()
    bias_d = nc.dram_tensor("bias", [128, 16], F32, kind="ExternalInput").ap()
    out_d = nc.dram_tensor("out", [BPC, 9], F32, kind="ExternalOutput").ap()
    with tile.TileContext(nc) as tc:
        _emit_core_program(tc, nc, pt_d, wpos_d, wm16_d, wmf_d, wmn_d,
                           bias_d, out_d, reps=reps)
    nc.compile()
    return nc


def prepare_host_inputs(inputs):
    """Shared (weights) and per-core (points) host-side packing."""
    import ml_dtypes
    BF = ml_dtypes.bfloat16
    f = lambda k: np.ascontiguousarray(np.asarray(inputs[k], np.float32))
    p = f("p")
    W_pos, b_pos = f("W_pos"), f("b_pos")
    W0, b0 = f("W0"), f("b0")
    W1, b1 = f("W1"), f("b1")
    W2, b2 = f("W2"), f("b2")
    W3, b3 = f("W3"), f("b3")
    Wc, bc = f("Wc"), f("bc")
    Wm0, bm0 = f("Wm0"), f("bm0")
    Wm1, bm1 = f("Wm1"), f("bm1")
    Wm2, bm2 = f("Wm2"), f("bm2")
    Wp, bp = f("Wp"), f("bp")

    wpos = np.broadcast_to(W_pos, (NQ, 3, 256)).copy()  # replicated per quad

    wm = np.zeros((128, WM_COLS), np.float32)
    blocks = [W0[:128], W0[128:], W1[:128], W1[128:], W2[:128], W2[128:],
              W3[:128], W3[128:], Wc, Wm0, Wm1, Wm2]
    for i, blk in enumerate(blocks):
        wm[:, 128 * i:128 * (i + 1)] = blk
    wm[:, 1536:1536 + 9] = Wp

    bias = np.zeros((128, 16), np.float32)
    bias[:, BC_BPOS_A] = b_pos[:128]
    bias[:, BC_BPOS_B] = b_pos[128:]
    bias[:, BC_B0] = b0
    bias[:, BC_B1] = b1
    bias[:, BC_B2] = b2
    bias[:, BC_B3] = b3
    bias[:, BC_BC] = bc
    bias[:, BC_BM0] = bm0
    bias[:, BC_BM1] = bm1
    bias[:, BC_BM2] = bm2
    bias[:9, BC_BP] = bp
    bias[:, BC_NEG_B0] = -b0
    bias[:, BC_NEG_B1] = -b1
    bias[:, BC_NEG_B2] = -b2
    bias[:, BC_NEG_B3] = -b3

    wmn = -np.concatenate([W1[128:], W2[128:], W3[128:]], axis=1)
    shared = {"wpos": wpos.astype(BF), "wm16": wm.astype(BF),
              "wmf": wm, "wmn": np.ascontiguousarray(wmn), "bias": bias}

    in_maps = []
    for core in range(NCORES):
        pc = p[core * BPC:(core + 1) * BPC]          # [BPC, T, 3]
        pt = np.empty((BPC, NQ, 3, QT), np.float32)
        for b in range(BPC):
            for q in range(NQ):
                pt[b, q] = pc[b, q * QT:(q + 1) * QT, :].T
        in_maps.append({"pt": pt.astype(BF), **shared})
    return in_maps


_PROGRAM_CACHE = {}


def kernel(**inputs):
    reps = 1
    if reps not in _PROGRAM_CACHE:
        _PROGRAM_CACHE[reps] = build_program(reps)
    nc = _PROGRAM_CACHE[reps]
    in_maps = prepare_host_inputs(inputs)
    res = run_bass_kernel_spmd(nc, in_maps, core_ids=list(range(NCORES)))
    out = np.concatenate([res.results[i]["out"] for i in range(NCORES)], axis=0)
    return out.astype(np.float32)
